# revision 20
# baseline (speedup 1.0000x reference)
"""FPS kernel v2 — engine-balanced distance passes + Pool-resident argmax tail.

Per batch per iteration, route r0 (batches 0, 2):
  DVE : U    = (x-cx)^2 + (y-cy)^2          SQDIFF2 custom
  DVE : D    = (z-cz)^2 + U                 SQADD custom
  DVE : dist = min(dist, D); pmax = rowmax  MINMAX custom (fused)
Route r1 (batches 1, 3):
  DVE : U    = SQDIFF2(x, y)
  Act : B    = Square(-z + cz) = (z-cz)^2
  Pool: D    = (U*1.0) + B                  scalar_tensor_tensor (exact)
  DVE : minmax as above
Cross-partition argmax (per group of GB=2 batches, NG=2 stagger groups)
runs entirely on Pool (gpsimd) so it never queues behind DVE big passes:
  M = allred_max(pmax); mpr = (pmax==M)*PmP (fused stt); R = allred_max;
  mrow = eq(mpr, R); PE gathers winning row (coords+scores);
  ohs = (score==M)*FmF (stt); F = allred_max(rowmax ohs); oht = eq(ohs,F);
  PE m2 broadcasts the centroid to bcp [128,3] PSUM.
Emit: one [1,GB] stt per group writes K = FREE*R + F directly at slot
(partition t%SLOTS, col (t//SLOTS)*nb + b); host decodes j = 33024 - K.
"""

import numpy as np
from contextlib import ExitStack

from concourse import bass, tile, mybir, bass_isa

f32 = mybir.dt.float32
i32 = mybir.dt.int32
Alu = mybir.AluOpType
Act = mybir.ActivationFunctionType
RO = bass_isa.ReduceOp

_OPS = {}


def register_fps_ops():
    if _OPS:
        return _OPS
    from concourse import dve_ops
    from concourse.dve_spec import (Spec, Src0, Src1, C0, C1, sq, minn, maxx,
                                    eq, lower)
    from concourse.dve_uop import DveOpSpec

    def make_op(name, spec):
        for op in dve_ops.OPS:
            if op.name == name:
                return op
        op = dve_ops.DveOp.__new__(dve_ops.DveOp)
        object.__setattr__(op, "name", name)
        object.__setattr__(op, "spec", spec)
        object.__setattr__(op, "subdim", False)
        object.__setattr__(op, "uops_sha", {})
        object.__setattr__(op, "perf_en", {})
        dve_ops.OPS.append(op)
        dve_ops.CUSTOM_DVE_SPECS[name] = spec
        dve_ops._SUB_OPCODE_FOR_NAME[name] = (
            dve_ops._CUSTOM_DVE_ROW_BASE + len(dve_ops.OPS) - 1
        )
        for ver in ("v3", "v4"):
            s = DveOpSpec(name=name, opcode=dve_ops.get_dve_sub_opcode(name),
                          uops=lower(spec, ver=ver),
                          rd1_en=dve_ops.has_src1(spec))
            op.uops_sha[ver] = s.sha(ver)
        return op

    def _accref(fn):
        def r(in0, in1, s0, s1, imm2):
            b = fn(in0, in1, s0, s1, imm2)
            return b, b.reshape(b.shape[0], -1).max(axis=-1, keepdims=True)
        return r

    _OPS["SQDIFF2"] = make_op("SQDIFF2_FPS", Spec(
        body=sq(Src0 - C0) + sq(Src1 - C1),
        reference=lambda in0, in1, s0, s1, imm2:
            (in0 - s0) * (in0 - s0) + (in1 - s1) * (in1 - s1),
    ))
    _OPS["SQADD"] = make_op("SQADD_FPS", Spec(
        body=sq(Src0 - C0) + Src1,
        reference=lambda in0, in1, s0, s1, imm2:
            (in0 - s0) * (in0 - s0) + in1,
    ))
    _OPS["MINMAX"] = make_op("MINMAX_FPS", Spec(
        body=minn(Src0, Src1), accum=maxx,
        reference=_accref(lambda in0, in1, s0, s1, imm2:
                          np.minimum(in0, in1)),
    ))
    _OPS["EQSCORE"] = make_op("EQSCORE_FPS", Spec(
        body=eq(Src0, C0) * Src1, accum=maxx,
        reference=_accref(lambda in0, in1, s0, s1, imm2:
                          (in0 == s0).astype(np.float32) * in1),
    ))
    return _OPS


def fps_ref_np(cloud: np.ndarray, npts: int) -> np.ndarray:
    B, N, _ = cloud.shape
    idx = np.zeros((B, npts), np.int64)
    for b in range(B):
        dist = np.full(N, 1e10, np.float32)
        far = 0
        for t in range(npts):
            idx[b, t] = far
            c = cloud[b, far]
            dx = cloud[b, :, 0] - c[0]
            dy = cloud[b, :, 1] - c[1]
            dz = cloud[b, :, 2] - c[2]
            d = (dx * dx + dy * dy) + dz * dz
            dist = np.minimum(dist, d)
            far = int(np.argmax(dist))
    return idx


def decode_k(kt: np.ndarray, nb: int, N: int, npts: int) -> np.ndarray:
    """kt flat [npts*nb] (K = FREE*R + F at position t*nb + b)
    -> indices [nb, npts] int64."""
    FREE = N // 128
    k = kt.reshape(npts, nb).T
    return (128 * FREE + FREE) - k.astype(np.int64)


ROUTES = ("r0", "r1", "r0", "r1")
NG = 4        # stagger groups
MAXU = 64     # iterations unrolled per hw-loop trip


def build_fps(tc, out_idx_d, pred, nb: int, N: int, NPTS: int,
              routes=ROUTES, ng=NG):
    ops = register_fps_ops()
    nc = tc.nc
    FREE = N // 128
    SLOTS = (NPTS + 127) // 128
    PMAX = NPTS // SLOTS
    W = min(FREE, 128)
    NCH = FREE // W
    GB = nb // ng
    assert 128 * FREE == N and NCH * W == FREE and PMAX * SLOTS == NPTS
    assert nb % ng == 0 and W == 128 and NCH == 2

    with ExitStack() as ctx:
        pool = ctx.enter_context(tc.tile_pool(name="main", bufs=1))
        psum = ctx.enter_context(tc.tile_pool(name="ps", bufs=2, space="PSUM"))

        C = [pool.tile([128, 3 * FREE], f32, name=f"C{b}") for b in range(nb)]
        dist = [pool.tile([128, FREE], f32, name=f"dist{b}") for b in range(nb)]
        U = [pool.tile([128, FREE], f32, name=f"U{b}") for b in range(nb)]
        B_ = [pool.tile([128, FREE], f32, name=f"B{b}")
              if routes[b] == "r1" else None for b in range(nb)]
        # Per-trip K staging on partition 0 (engine ops must start at a
        # base partition): K_b of iteration t at col (t - trip_base)*nb + b.
        # Flushed to DRAM once per unrolled trip (runtime-offset DMA —
        # runtime offsets on ENGINE ops mis-lower, DMA offsets are fine).
        RPT = MAXU // SLOTS
        wr = [pool.tile([128, 3 * NCH], f32, name=f"wr{b}") for b in range(nb)]
        bcp = [psum.tile([128, 3], f32, tag=f"bcp{b}", name=f"bcp{b}", bufs=1)
               for b in range(nb)]
        # SBUF copy of the z centroid component (Act bias must read SBUF)
        bcs = [pool.tile([128, 1], f32, name=f"bcs{b}")
               if routes[b] == "r1" else None for b in range(nb)]

        pmax4 = [pool.tile([128, GB], f32, name=f"pmax4g{g}") for g in range(ng)]
        M4 = [pool.tile([128, GB], f32, name=f"M4g{g}") for g in range(ng)]
        mpr4 = [pool.tile([128, GB], f32, name=f"mpr4g{g}") for g in range(ng)]
        R4 = [pool.tile([128, GB], f32, name=f"R4g{g}") for g in range(ng)]
        F4 = [pool.tile([128, GB], f32, name=f"F4g{g}") for g in range(ng)]
        mrow4 = [pool.tile([128, GB], f32, name=f"mrow4g{g}") for g in range(ng)]
        ohs = [pool.tile([128, NCH * GB], f32, name=f"ohsg{g}") for g in range(ng)]
        oht = [pool.tile([128, NCH * GB], f32, name=f"ohtg{g}") for g in range(ng)]
        ohsacc = [pool.tile([128, GB], f32, name=f"ohsaccg{g}") for g in range(ng)]

        PmP = pool.tile([128, 1], f32)
        ptmp = pool.tile([128, 1], i32)
        FmF = pool.tile([128, NCH], f32)
        fitmp = pool.tile([128, NCH], i32)

        # ---- constants / init ----
        nc.gpsimd.iota(ptmp[:], [[1, 1]], base=0, channel_multiplier=1)
        nc.vector.tensor_copy(PmP[:], ptmp[:])
        nc.vector.tensor_scalar(PmP[:], PmP[:], -1.0, 128.0, Alu.mult, Alu.add)
        nc.gpsimd.iota(fitmp[:], [[W, NCH]], base=0, channel_multiplier=1)
        nc.vector.tensor_copy(FmF[:], fitmp[:])
        nc.vector.tensor_scalar(FmF[:], FmF[:], -1.0, float(FREE), Alu.mult,
                                Alu.add)
        for g in range(ng):
            nc.any.memset(R4[g][:], 128.0)        # encodes j=0: p*=0
            nc.any.memset(F4[g][:], float(FREE))  # encodes j=0: fi=0
            nc.any.memset(mrow4[g][:], 0.0)
            nc.any.memset(mrow4[g][0:1, :], 1.0)
            nc.any.memset(oht[g][:], 0.0)
            for bl in range(GB):
                nc.any.memset(oht[g][0:1, NCH * bl:NCH * bl + 1], 1.0)
        for b in range(nb):
            nc.any.memset(dist[b][:], 1e10)

        # Contiguous load + on-chip de-interleave (12B-stride DMA is ~15x
        # slower than contiguous).
        craw = [pool.tile([128, 3 * FREE], f32, tag="craw", name=f"craw{b}",
                          bufs=2) for b in range(nb)]
        for b in range(nb):
            nc.sync.dma_start(craw[b][:, :], pred[b:b + 1, :, :])
            cr3 = craw[b].rearrange("p (f c) -> p c f", c=3)
            for c in range(3):
                nc.any.tensor_copy(C[b][:, c * FREE:(c + 1) * FREE], cr3[:, c, :])

        def m1(b, score_src):
            g, bl = b // GB, b % GB
            wps = psum.tile([128, 3 * NCH + NCH], f32, tag="wps", name=f"wps{b}")
            for k in range(3 * NCH):
                nc.tensor.matmul(
                    wps[0:W, k:k + 1], C[b][:, W * k:W * (k + 1)],
                    mrow4[g][:, bl:bl + 1], start=True, stop=True,
                )
            for h in range(NCH):
                nc.tensor.matmul(
                    wps[0:W, 3 * NCH + h:3 * NCH + h + 1],
                    score_src[b][:, W * h:W * (h + 1)],
                    mrow4[g][:, bl:bl + 1], start=True, stop=True,
                )
            # coords PSUM -> SBUF for m2's rhs (Act is idle; off the tail)
            nc.scalar.activation(wr[b][0:W, 0:3 * NCH], wps[0:W, 0:3 * NCH],
                                 Act.Copy)
            return wps

        def m2(b):
            g, bl = b // GB, b % GB
            for h in range(NCH):
                ohb = oht[g][0:W, NCH * bl + h:NCH * bl + h + 1]
                nc.tensor.matmul(
                    bcp[b][:, :], ohb.broadcast_to([W, 128]),
                    wr[b][0:W, h:3 * NCH:NCH],
                    start=(h == 0), stop=(h == NCH - 1),
                )
            if routes[b] == "r1":
                nc.scalar.activation(bcs[b][:, 0:1], bcp[b][:, 2:3], Act.Copy)

        for b in range(nb):
            m1(b, dist)
            m2(b)

        V = nc.vector
        G = nc.gpsimd
        S = nc.scalar

        def body(stage, jj, ct):
            for g in range(ng):
                # emit previous winner K = FREE*R + F; host decodes
                # j = 128*FREE + FREE - K
                o = (jj * SLOTS + ct) * nb + g * GB
                V.scalar_tensor_tensor(
                    stage[0:1, o:o + GB],
                    R4[g][0:1, :], float(FREE), F4[g][0:1, :],
                    Alu.mult, Alu.add,
                )
                bs = list(range(g * GB, (g + 1) * GB))
                for b in bs:
                    bl = b % GB
                    X = C[b][:, 0 * FREE:1 * FREE]
                    Y = C[b][:, 1 * FREE:2 * FREE]
                    Z = C[b][:, 2 * FREE:3 * FREE]
                    if routes[b] == "r0":
                        V._custom_dve(ops["SQDIFF2"], out=U[b][:], in0=X, in1=Y,
                                      s0=bcp[b][:, 0:1], s1=bcp[b][:, 1:2])
                        V._custom_dve(ops["SQADD"], out=U[b][:], in0=Z,
                                      in1=U[b][:], s0=bcp[b][:, 2:3])
                    else:  # r1
                        V._custom_dve(ops["SQDIFF2"], out=U[b][:], in0=X, in1=Y,
                                      s0=bcp[b][:, 0:1], s1=bcp[b][:, 1:2])
                        S.activation(B_[b][:], Z, Act.Square,
                                     bias=bcs[b][:, 0:1], scale=-1.0)
                        G.tensor_tensor(U[b][:], U[b][:], B_[b][:], Alu.add)
                    # dist = min(dist, D); pmax = rowmax (custom DVE)
                    V._custom_dve(ops["MINMAX"], out=dist[b][:],
                                  in0=dist[b][:], in1=U[b][:],
                                  accum_out=pmax4[g][:, bl:bl + 1])
                # cross-partition winner on Pool: max value, smallest
                # partition on ties
                G.partition_all_reduce(M4[g][:, :], pmax4[g][:, :],
                                       channels=128, reduce_op=RO.max)
                for bl in range(GB):
                    V._custom_dve(ops["EQSCORE"],
                                  out=mpr4[g][:, bl:bl + 1],
                                  in0=pmax4[g][:, bl:bl + 1],
                                  in1=PmP[:, :],
                                  s0=M4[g][:, bl:bl + 1])
                G.partition_all_reduce(R4[g][:, :], mpr4[g][:, :],
                                       channels=128, reduce_op=RO.max)
                for bl in range(GB):
                    G.tensor_scalar(mrow4[g][:, bl:bl + 1],
                                    mpr4[g][:, bl:bl + 1],
                                    R4[g][:, bl:bl + 1], None, Alu.is_equal)
                # winning row (coords + dist) via PE; within-row first max
                for b in bs:
                    bl = b % GB
                    wps = m1(b, dist)
                    G.scalar_tensor_tensor(
                        ohs[g][0:W, NCH * bl:NCH * bl + NCH],
                        wps[0:W, 3 * NCH:3 * NCH + NCH],
                        M4[g][0:W, bl:bl + 1], FmF[0:W, :],
                        Alu.is_equal, Alu.mult)
                    G.tensor_tensor(
                        ohsacc[g][0:W, bl:bl + 1],
                        ohs[g][0:W, NCH * bl:NCH * bl + 1],
                        ohs[g][0:W, NCH * bl + 1:NCH * bl + NCH],
                        Alu.max)
                G.partition_all_reduce(F4[g][:, :], ohsacc[g][:, :],
                                       channels=128, reduce_op=RO.max)
                for bl in range(GB):
                    G.tensor_scalar(
                        oht[g][0:W, NCH * bl:NCH * bl + NCH],
                        ohs[g][0:W, NCH * bl:NCH * bl + NCH],
                        F4[g][0:W, bl:bl + 1], None, Alu.is_equal,
                    )
                for b in bs:
                    m2(b)

        assert NPTS % SLOTS == 0

        def unrollable_body(rv0, unroll):
            stage = pool.tile([1, RPT * SLOTS * nb], f32, tag="stage",
                              name="stage", bufs=2)
            for j in range(unroll):
                for s in range(SLOTS):
                    body(stage, j, s)
            nc.sync.dma_start(
                out_idx_d[:, bass.DynSlice(rv0 * SLOTS * nb,
                                           unroll * SLOTS * nb)],
                stage[:, 0:unroll * SLOTS * nb])

        tc.For_i_unrolled_general(
            0, PMAX, 1, unrollable_body, max_unroll=MAXU // SLOTS,
            hint_engines=(mybir.EngineType.PE, mybir.EngineType.Activation,
                          mybir.EngineType.Pool, mybir.EngineType.DVE),
        )


# ----------------------------------------------------------------------------
# Self-contained kernel entry point: full inputs in, full outputs out.
# ----------------------------------------------------------------------------

NB = 4          # batches per core
N_PTS = 32768   # points per cloud
NPTS_OUT = 1024
NCORES = 8
SLOTS_OUT = NPTS_OUT // 128 * 0 + (NPTS_OUT + 127) // 128
PMAX_OUT = NPTS_OUT // SLOTS_OUT

_NC_CACHE = {}


def _get_nc():
    if "nc" in _NC_CACHE:
        return _NC_CACHE["nc"]
    from concourse import bacc, tile as _tile

    nc = bacc.Bacc("TRN2", target_bir_lowering=False, debug=False)
    pred = nc.dram_tensor(
        "pred_cloud", [NB, N_PTS, 3], mybir.dt.float32, kind="ExternalInput"
    ).ap()
    out = nc.dram_tensor(
        "out", [1, NPTS_OUT * NB], mybir.dt.float32, kind="ExternalOutput"
    ).ap()
    with _tile.TileContext(nc) as tc:
        build_fps(tc, out, pred, NB, N_PTS, NPTS_OUT)
    nc.compile()
    _NC_CACHE["nc"] = nc
    return nc


def kernel(pred_cloud):
    """pred_cloud [32, 32768, 3] f32 -> sampled points [32, 1024, 3] f32."""
    from concourse import bass_utils

    pred_cloud = np.ascontiguousarray(np.asarray(pred_cloud, dtype=np.float32))
    assert pred_cloud.shape == (NB * NCORES, N_PTS, 3)
    nc = _get_nc()
    in_maps = [
        {"pred_cloud": np.ascontiguousarray(pred_cloud[NB * i:NB * (i + 1)])}
        for i in range(NCORES)
    ]
    res = bass_utils.run_bass_kernel_spmd(nc, in_maps, core_ids=list(range(NCORES)))
    idx = np.concatenate(
        [decode_k(np.asarray(res.results[i]["out"], dtype=np.float64)
                  .reshape(NPTS_OUT * NB).astype(np.int64),
                  NB, N_PTS, NPTS_OUT)
         for i in range(NCORES)], axis=0
    )  # [32, 1024] int64
    out = np.take_along_axis(pred_cloud, idx[:, :, None], axis=1)
    return np.ascontiguousarray(out.astype(np.float32))
